# revision 22
# baseline (speedup 1.0000x reference)
"""Causal self-attention (QK-RMSNorm + RoPE) Trainium2 kernel.

Sharding (Megatron-style, per the TP-over-heads hint):
  8 cores = 2 (batch) x 4 (head groups of 4 heads).
  Each core computes qkv/attention for its 4 heads on its batch and a partial
  projection output; the host sums the 4 partials per batch (the "all-reduce")
  and transposes (the device emits the output feature-major).

Schedule: quarter-interleaved software pipeline. Phase 1 (QKV, PE-saturated)
is chopped into 4 token-quarters of (v, q, k) sub-passes; the attention +
projection for query-block j (ACT/DVE-heavy: exp, denominator, normalize) is
woven chunk-by-chunk into quarter j+1's sub-passes so every engine stays fed.

Per-core pipeline (all matmuls bf16 with fp32 PSUM accumulation):
  qkv: x cast-loaded f32->bf16 (SWDGE), PE-transposed to x^T; qkv = W @ x^T
       token-major; q/k: RMSNorm stats on ACT, 5-op RoPE on DVE with the
       1/rms scale folded in after the rotation (tables pre-hold the norm
       weights: [cos*w1, cos*w2, -sin*w2, sin*w1]); PE head-transpose to the
       [d, t] attention layout.
  attention: exact-causal; full k-tiles unmasked, 4 shrinking diagonal blocks
       (512/384/256/128) with a single [128,128] tril mask on DVE; exp on ACT
       (no max subtraction: |scores| <= sqrt(hd)); denominator in bf16 via
       DVE adds + one M=128 bf16 ones-matmul (broadcasts the partition
       reduction) + fast approximate reciprocal.
"""

import math
from contextlib import ExitStack

import numpy as np
import ml_dtypes

import concourse.bass as bass
import concourse.mybir as mybir
import concourse.tile as tile
from concourse import bacc

F32 = mybir.dt.float32
BF16 = mybir.dt.bfloat16
AF = mybir.ActivationFunctionType

# Problem constants (hardcoded; kernel.py must be self-contained)
B, T, C, H, HD = 2, 2048, 2048, 16, 128
N_CORES = 8
DP = 2                 # data-parallel ways (batch)
TPW = N_CORES // DP    # tensor-parallel ways (head groups)
HG = H // TPW          # heads per core
EPS = 1e-6


def build_nc(T_=T, C_=C, HG_=HG, hd=HD, TQ=512):
    NT = T_ // 128       # token tiles
    NCt = C_ // 128      # contraction tiles for qkv
    NJ = T_ // TQ        # query-block tiles (= quarters)
    NO = C_ // 128       # output feature tiles
    R = TQ // 128        # 128-tiles per query block (diagonal blocks)
    F1 = HG_ * hd        # width of one of q/k/v chunks on this core
    HB = hd // 2
    QT = NT // NJ        # token tiles per quarter
    sm_scale = 1.0 / math.sqrt(hd)

    nc = bacc.Bacc(None, target_bir_lowering=False)
    x = nc.dram_tensor("x", [T_, C_], F32, kind="ExternalInput")
    wqkvT = nc.dram_tensor("wqkvT", [C_, 3 * F1], BF16, kind="ExternalInput")
    wprojT = nc.dram_tensor("wprojT", [F1, C_], BF16, kind="ExternalInput")
    rope_q = nc.dram_tensor("rope_q", [T_, 4 * HB], BF16, kind="ExternalInput")
    rope_k = nc.dram_tensor("rope_k", [T_, 4 * HB], BF16, kind="ExternalInput")
    masks_d = nc.dram_tensor("masks", [128, 128], BF16, kind="ExternalInput")
    ident_d = nc.dram_tensor("ident", [128, 128], BF16, kind="ExternalInput")
    outT = nc.dram_tensor("outT", [C_, T_], F32, kind="ExternalOutput")

    with tile.TileContext(nc) as tc, ExitStack() as big:
        persist = big.enter_context(tc.tile_pool(name="persist", bufs=1))
        sb = big.enter_context(tc.tile_pool(name="sb", bufs=2))
        scr = big.enter_context(tc.tile_pool(name="scr", bufs=3))
        pP = big.enter_context(tc.tile_pool(name="pP", bufs=12))
        # PSUM: 2 (work512 ring) + 1 (yps) + 3 (scores) + 2 (transposes) = 8
        wps = big.enter_context(tc.tile_pool(name="wps", bufs=2, space="PSUM"))
        yps_p = big.enter_context(tc.tile_pool(name="yps", bufs=1, space="PSUM"))
        sps = big.enter_context(tc.tile_pool(name="sps", bufs=3, space="PSUM"))
        tps = big.enter_context(tc.tile_pool(name="tps", bufs=2, space="PSUM"))

        v_all = persist.tile([128, NT, F1], BF16, tag="v_all")
        qkT = persist.tile([128, 2, HG_, T_], BF16, tag="qkT")
        wp = persist.tile([128, HG_, C_], BF16, tag="wp")
        ones_b = persist.tile([128, 128], BF16, tag="ones_b")
        nc.vector.memset(ones_b, 1.0)
        eps_t = persist.tile([128, 1], F32, tag="eps")
        nc.vector.memset(eps_t, EPS)
        tril = persist.tile([128, 128], BF16, tag="tril")
        nc.sync.dma_start(tril, masks_d[:])
        ident = persist.tile([128, 128], BF16, tag="ident")
        nc.sync.dma_start(ident, ident_d[:])
        # prewarm the ACT tables (Square/Sqrt/Exp) during the DMA-bound start
        warm = persist.tile([128, 1], F32, tag="warm")
        nc.scalar.activation(warm, eps_t, AF.Square)
        nc.scalar.activation(warm, eps_t, AF.Sqrt)
        nc.scalar.activation(warm, eps_t, AF.Exp)

        rope_sb = {}
        xhs = [None] * NT

        def load_rope(nm, dr):
            t_ = persist.tile([128, NT, 4 * HB], BF16, tag=f"rope{nm}")
            nc.gpsimd.dma_start(t_, dr[:].rearrange("(n p) f -> p n f", p=128))
            rope_sb[nm] = t_

        def load_wt(fc):
            wt = []
            for ci in range(NCt):
                t_ = sb.tile([128, F1], BF16, tag=f"wt{ci}", bufs=2)
                nc.sync.dma_start(
                    t_, wqkvT[ci * 128:(ci + 1) * 128, fc * F1:(fc + 1) * F1])
                wt.append(t_)
            return wt

        def v_chunk(i, wt):
            # stage + transpose x tile, then the v matmul chain
            xb = sb.tile([128, C_], BF16, tag="stage", bufs=2)
            nc.gpsimd.dma_start(xb, x[i * 128:(i + 1) * 128, :])
            xhi = sb.tile([128, NCt, 128], BF16, tag=f"xh{i % 8}", bufs=1)
            for cq in range(NCt // 4):
                pt = tps.tile([128, 512], BF16, tag="pstr")
                for c4 in range(4):
                    ci = 4 * cq + c4
                    nc.tensor.transpose(
                        pt[:, c4 * 128:(c4 + 1) * 128],
                        xb[:, ci * 128:(ci + 1) * 128], ident)
                nc.vector.tensor_copy(xhi[:, 4 * cq:4 * cq + 4, :], pt)
            xhs[i] = xhi
            ps = wps.tile([128, F1], F32, tag="w512")
            for ci in range(NCt):
                nc.tensor.matmul(ps, xhi[:, ci, :], wt[ci],
                                 start=(ci == 0), stop=(ci == NCt - 1))
            nc.scalar.copy(v_all[:, i, :], ps)

        def qk_chunk(i, fc, wt):
            ps = wps.tile([128, F1], F32, tag="w512")
            for ci in range(NCt):
                nc.tensor.matmul(ps, xhs[i][:, ci, :], wt[ci],
                                 start=(ci == 0), stop=(ci == NCt - 1))
            rp = rope_sb["q" if fc == 0 else "k"]
            # RMS norm stats in fp32 off PSUM (ACT: square+accum, sqrt)
            sq = scr.tile([128, F1], BF16, tag="sq")
            ssq = scr.tile([128, HG_], F32, tag="ssq")
            for h in range(HG_):
                nc.scalar.activation(
                    sq[:, h * hd:(h + 1) * hd], ps[:, h * hd:(h + 1) * hd],
                    AF.Square, accum_out=ssq[:, h:h + 1])
            sstd = scr.tile([128, HG_], F32, tag="sstd")
            nc.scalar.activation(
                sstd, ssq, AF.Sqrt, bias=eps_t[:, 0:1], scale=1.0 / hd)
            rinv = scr.tile([128, HG_], F32, tag="rinv")
            nc.vector.reciprocal(rinv, sstd)
            # 5-op RoPE from PSUM with the rms scale folded in afterwards.
            # tables per token: [A=cos*w1 | Ct=cos*w2 | Bn=-sin*w2 | D=sin*w1]
            # rq[h,0,:] = (ps[h,0]*A + ps[h,1]*Bn) * rinv[h]
            # rq[h,1,:] = (ps[h,1]*Ct + ps[h,0]*D) * rinv[h]
            ps3 = ps.rearrange("p (h two d) -> p h two d", h=HG_, two=2)

            def tab(c_idx, width):
                bse = rp[:, i, c_idx * HB:c_idx * HB + width]
                return bass.AP(
                    tensor=bse.tensor, offset=bse.offset,
                    ap=[list(bse.ap[0]), [0, HG_], list(bse.ap[-1])])

            u = scr.tile([128, F1], BF16, tag="u")
            u3 = u.rearrange("p (h two d) -> p h two d", h=HG_, two=2)
            w_ = scr.tile([128, F1], BF16, tag="w_")
            w3 = w_.rearrange("p (h two d) -> p h two d", h=HG_, two=2)
            nc.vector.tensor_mul(
                u.rearrange("p (h td) -> p h td", h=HG_),
                ps.rearrange("p (h td) -> p h td", h=HG_),
                tab(0, 2 * HB))
            nc.vector.tensor_mul(w3[:, :, 0, :], ps3[:, :, 1, :], tab(2, HB))
            nc.vector.tensor_mul(w3[:, :, 1, :], ps3[:, :, 0, :], tab(3, HB))
            rq = scr.tile([128, F1], BF16, tag="rq")
            nc.vector.tensor_add(rq, u, w_)
            rqs = scr.tile([128, F1], BF16, tag="rqs")
            rinv_b = bass.AP(
                tensor=rinv.tensor, offset=rinv.offset,
                ap=[list(rinv.ap[0]), list(rinv.ap[-1]), [0, 2 * HB]])
            nc.vector.tensor_mul(
                rqs.rearrange("p (h td) -> p h td", h=HG_),
                rq.rearrange("p (h td) -> p h td", h=HG_), rinv_b)
            # head-transpose q/k (4 heads into one PSUM tile, 1 copy)
            pt = tps.tile([128, 512], BF16, tag="pstr")
            for h in range(HG_):
                nc.tensor.transpose(
                    pt[:, h * hd:(h + 1) * hd],
                    rqs[:, h * hd:(h + 1) * hd], ident)
            ptv = pt.rearrange("p (h t) -> p h t", h=HG_)
            nc.vector.tensor_copy(qkT[:, fc, :, i * 128:(i + 1) * 128], ptv)

        def attn_head(j, h):
            nfull = R * j
            dacc = scr.tile([128, TQ], BF16, tag="dacc", bufs=2)
            plist = []
            qs = qkT[:, 0, h, j * TQ:(j + 1) * TQ]
            # full (unmasked) k tiles: score -> exp -> denominator add
            for k in range(nfull):
                s1 = sps.tile([128, TQ], F32, tag="s1")
                nc.tensor.matmul(
                    s1, qkT[:, 1, h, k * 128:(k + 1) * 128], qs,
                    start=True, stop=True)
                p1 = pP.tile([128, TQ], BF16, tag="p2")
                nc.scalar.activation(p1, s1, AF.Exp, scale=sm_scale)
                if k == 0:
                    nc.vector.tensor_copy(dacc, p1)
                else:
                    nc.vector.tensor_add(dacc, dacc, p1)
                plist.append(p1)
            # diagonal: 4 blocks of shrinking width; first 128 cols of each
            # block are the true diagonal -> tril mask on DVE
            pds = []
            for d in range(R):
                qo = 128 * d
                sd = sps.tile([128, TQ], F32, tag="s1")
                nc.tensor.matmul(
                    sd[:, 0:TQ - qo],
                    qkT[:, 1, h, (R * j + d) * 128:(R * j + d + 1) * 128],
                    qkT[:, 0, h, j * TQ + qo:(j + 1) * TQ],
                    start=True, stop=True)
                pd = pP.tile([128, TQ - qo], BF16, tag=f"pd{d}", bufs=2)
                nc.scalar.activation(pd, sd[:, 0:TQ - qo], AF.Exp,
                                     scale=sm_scale)
                nc.vector.tensor_mul(pd[:, 0:128], pd[:, 0:128], tril)
                if d == 0 and nfull == 0:
                    nc.vector.tensor_copy(dacc, pd)
                else:
                    nc.vector.tensor_add(
                        dacc[:, qo:TQ], dacc[:, qo:TQ], pd)
                pds.append(pd)
            # y^T accumulation: full tiles then ragged diagonal
            yps = yps_p.tile([128, TQ], F32, tag="yps")
            for k in range(nfull):
                nc.tensor.matmul(
                    yps, v_all[:, k, h * hd:(h + 1) * hd], plist[k],
                    start=(k == 0), stop=False)
            for d in range(R):
                qo = 128 * d
                nc.tensor.matmul(
                    yps[:, qo:TQ],
                    v_all[:, R * j + d, h * hd:(h + 1) * hd], pds[d],
                    start=(nfull == 0 and d == 0), stop=(d == R - 1),
                    skip_group_check=True)
            # partition-dim denominator reduction via bf16 ones-matmul;
            # M=128 broadcasts the row sum to every output partition
            psr = wps.tile([128, TQ], F32, tag="w512")
            nc.tensor.matmul(psr, ones_b, dacc, start=True, stop=True)
            rb = scr.tile([128, TQ], F32, tag="rb", bufs=2)
            nc.vector.reciprocal_approx_fast(rb, psr)
            yT = yTs[j % 2]
            nc.vector.tensor_mul(yT[:, h, :], yps, rb)

        def proj_half(j, half):
            yT = yTs[j % 2]
            for o in range(half * NO // 2, (half + 1) * NO // 2):
                pp = wps.tile([128, TQ], F32, tag="w512")
                for ci in range(HG_):
                    nc.tensor.matmul(
                        pp, wp[:, ci, o * 128:(o + 1) * 128], yT[:, ci, :],
                        start=(ci == 0), stop=(ci == HG_ - 1))
                ost = sb.tile([128, TQ], F32, tag="ost", bufs=4)
                if o % 2 == 0:
                    nc.scalar.copy(ost, pp)
                else:
                    nc.vector.tensor_copy(ost, pp)
                nc.scalar.dma_start(
                    outT[o * 128:(o + 1) * 128, j * TQ:(j + 1) * TQ], ost)

        yTs = []
        for m in range(2):
            yT_m = persist.tile([128, HG_, TQ], BF16, tag=f"yT{m}")
            yTs.append(yT_m)

        for qt in range(NJ):
            j = qt - 1  # attention block woven into this quarter
            tiles = list(range(qt * QT, (qt + 1) * QT))
            chunks = []
            wt_v = load_wt(2)
            for i in tiles:
                chunks.append(("v", i, wt_v))
            if qt == 0:
                load_rope("q", rope_q)
                # Wproj loads after the startup-critical x tiles (SWDGE ring)
                for ci in range(HG_):
                    nc.gpsimd.dma_start(
                        wp[:, ci, :], wprojT[ci * 128:(ci + 1) * 128, :])
            wt_q = load_wt(0)
            for i in tiles:
                chunks.append(("q", i, wt_q))
            if qt == 0:
                load_rope("k", rope_k)
            wt_k = load_wt(1)
            for i in tiles:
                chunks.append(("k", i, wt_k))

            if qt == 0:
                for kind, i, wt in chunks:
                    if kind == "v":
                        v_chunk(i, wt)
                    else:
                        qk_chunk(i, 0 if kind == "q" else 1, wt)
            else:
                # weave attention(j) head/proj chunks between pass chunks
                attn_chunks = ([("h", hh) for hh in range(HG_)]
                               + [("p", 0), ("p", 1)])
                ai = 0
                for n, (kind, i, wt) in enumerate(chunks):
                    if kind == "v":
                        v_chunk(i, wt)
                    else:
                        qk_chunk(i, 0 if kind == "q" else 1, wt)
                    if n % 2 == 1 and ai < len(attn_chunks):
                        ak, av = attn_chunks[ai]
                        ai += 1
                        if ak == "h":
                            attn_head(j, av)
                        else:
                            proj_half(j, av)
                while ai < len(attn_chunks):
                    ak, av = attn_chunks[ai]
                    ai += 1
                    if ak == "h":
                        attn_head(j, av)
                    else:
                        proj_half(j, av)

        # tail: last attention block + projection (no overlap partner)
        for hh in range(HG_):
            attn_head(NJ - 1, hh)
        proj_half(NJ - 1, 0)
        proj_half(NJ - 1, 1)

    nc.compile()
    return nc


def make_host_inputs(x, Wqkv, Wproj, q_norm_w, k_norm_w, rope_cos, rope_sin,
                     T_=T, C_=C, HG_=HG, hd=HD):
    """Build the 8 per-core input maps (sharding done on host)."""
    H_ = Wqkv.shape[0] // (3 * hd)
    tpw = H_ // HG_
    HB = hd // 2

    def rope_tables(w):
        # rq1 = qn1*(cos*w1) + qn2*(-sin*w2); rq2 = qn2*(cos*w2) + qn1*(sin*w1)
        w1, w2 = w[:HB], w[HB:]
        A = rope_cos * w1[None, :]
        Ct = rope_cos * w2[None, :]
        Bn = -(rope_sin * w2[None, :])
        D = rope_sin * w1[None, :]
        return np.ascontiguousarray(
            np.concatenate([A, Ct, Bn, D], axis=1).astype(ml_dtypes.bfloat16)
        )

    rope_q_h = rope_tables(np.asarray(q_norm_w, dtype=np.float32))
    rope_k_h = rope_tables(np.asarray(k_norm_w, dtype=np.float32))

    # single within-tile diagonal causal mask: valid when tk <= tq
    tk = np.arange(128)[:, None]
    tq = np.arange(128)[None, :]
    tril_m = (tk <= tq).astype(ml_dtypes.bfloat16)

    Wqkv = np.asarray(Wqkv, dtype=np.float32)
    Wproj = np.asarray(Wproj, dtype=np.float32)
    x = np.asarray(x, dtype=np.float32)

    in_maps = []
    for core in range(N_CORES):
        b = core // tpw
        g = core % tpw
        rs = slice(g * HG_ * hd, (g + 1) * HG_ * hd)
        W_shard = np.concatenate(
            [Wqkv[0 * H_ * hd:][rs.start:rs.stop],
             Wqkv[1 * H_ * hd:][rs.start:rs.stop],
             Wqkv[2 * H_ * hd:][rs.start:rs.stop]], axis=0
        )  # [3*F1, C]
        in_maps.append({
            "ident": np.eye(128, dtype=ml_dtypes.bfloat16),
            "x": np.ascontiguousarray(x[b]),
            "wqkvT": np.ascontiguousarray(W_shard.T).astype(ml_dtypes.bfloat16),
            "wprojT": np.ascontiguousarray(Wproj[:, rs].T).astype(ml_dtypes.bfloat16),
            "rope_q": rope_q_h,
            "rope_k": rope_k_h,
            "masks": tril_m,
        })
    return in_maps


_NC_CACHE = {}


def run_spmd(inputs, **run_kwargs):
    from concourse.bass_utils import run_bass_kernel_spmd

    x = np.asarray(inputs["x"])
    in_maps = make_host_inputs(
        x, inputs["Wqkv"], inputs["Wproj"], inputs["q_norm_w"],
        inputs["k_norm_w"], inputs["rope_cos"], inputs["rope_sin"],
    )
    if "nc" not in _NC_CACHE:
        _NC_CACHE["nc"] = build_nc()
    nc = _NC_CACHE["nc"]
    res = run_bass_kernel_spmd(nc, in_maps, core_ids=list(range(N_CORES)),
                               **run_kwargs)
    tpw = N_CORES // B
    out = np.zeros((B, T, C), dtype=np.float32)
    for core in range(N_CORES):
        b = core // tpw
        out[b] += res.results[core]["outT"].T
    return out, res


def kernel(**inputs):
    out, _ = run_spmd(inputs)
    return out
